# revision 25
# baseline (speedup 1.0000x reference)
"""CoarseWarp Trainium2 kernel (v5: multi-queue pipelined gather + fold).

Reference computation (shapes hardcoded):
  lr [2,64,64,64] (shape-only), ref [2,64,254,254], index_map [2,64516]
  padded = reflect-pad(ref, 1)                      # [2,64,256,256]
  (yp, xp) = divmod(index_map[b, y*254+x], 254)
  out[b,c,y+i,x+j] += padded[b,c,yp+i,xp+j]  (i,j in 0..2)
  out: [2,64,256,256] f32

Strategy (8 NeuronCores, pure data parallel):
  core k handles b = k//4, output rows [ (k%4)*64, +64 ).

  Host builds, per batch, a fp16 patch table anchored at even x':
    T[1 + r*127 + e] = padded[r:r+3, 2e:2e+4, :]    # [i(3), u(4), c(64)]
  (entry 0 = zeros, used to pad streams) so each output position l
  needs ONE 1536 B dma_gather element: idx = 1 + yp*127 + xp//2, and
  the intra-element x-offset is phi = xp%2, supplied as a dense fp16
  mask stream.  Tap j reads u = phi + j, realized on-chip as 3
  copy_predicated selects (u[v] := phi ? u[v+1] : u[v]).

  The gather is descriptor-generation-bound on the Q7 SWDGE path
  (~8 ns/descriptor per cpu pair), so the 4608 indices per 16-row
  block are split into nine 512-index dma_gathers round-robined over
  4 SWDGE queues (4 Q7 cpu pairs generating in parallel, sharing the
  16 SDMA engines).  G4 is triple-buffered so descriptor generation
  runs ~2 blocks ahead of the SDMA transfers and the HBM random-read
  stream (~265 GB/s effective) never idles.  Per block the
  compaction+fold is issued in two column-halves so DVE/PE start on
  the first 5 sub-gathers while the rest are still in flight.

  The fold is deterministic: gathered slot x lands at partition x%128,
  col x//128; contribution (i, v) adds G[x][i, v] into out[y+i, x+v].
  Per 16-row block the fold runs on the (otherwise idle) Tensor
  engine: the x+v partition shift is a banded stationary matrix
  (slices of one host-built [128,264] fp16 tile: shift-by-v plus a
  wrap matrix for the 128-boundary crossing), and the 9 (i,v) terms
  accumulate in PSUM (f32) per 512-element chunk.  Scalar copies
  PSUM->SBUF and the result is written out sequentially (16 KB
  descriptors, X-major HBM layout that the host assembles back).
"""

import numpy as np

B, C = 2, 64
HR = WR = 254
HO = WO = 256
L = HR * WR            # 64516
NTAB = 1 + HR * 127    # 32259 table entries (entry 0 = zeros)
ES = 768               # fp16 elems per entry: 3*4*64 (1536 B)
N_CORES = 8
CORES_PER_LAUNCH = 8
SLAB = 64              # output rows per core
NBLK = 4
NY = SLAB // NBLK      # 16 output rows per block
NG = NY + 2            # 18 gathered l-rows per block
NCOL = NG * 2          # 36 G4 cols per block
NIDX_BLK = NG * 256    # 4608 gather indices per block
SUBI = 512             # indices per SWDGE sub-gather (9 per block)
NQ = 4                 # SWDGE queues (desc-gen Q7 cpu pairs)


def _wrap16(stream: np.ndarray) -> np.ndarray:
    """Index stream [N] (N%16==0) -> ucode layout [128, N//16] int16."""
    a = stream.reshape(-1, 16).T.astype(np.int16)
    return np.tile(a, (8, 1))


def _build_tables(ref: np.ndarray) -> list[np.ndarray]:
    """Per-batch fp16 patch tables [NTAB, ES]."""
    padded = np.pad(ref.astype(np.float32),
                    ((0, 0), (0, 0), (1, 1), (1, 1)), mode="reflect")
    tabs = []
    for b in range(B):
        p = np.ascontiguousarray(
            padded[b].transpose(1, 2, 0)).astype(np.float16)  # [256,256,64]
        T = np.zeros((NTAB, ES), np.float16)
        V = T[1:].reshape(HR, 127, 3, 4, C)
        for i in range(3):
            for u in range(4):
                V[:, :, i, u, :] = p[i:i + HR, u:u + 254:2, :][:, :127, :]
        tabs.append(T)
    return tabs


def _build_streams(index_map: np.ndarray):
    """Per-core gather idx [128, NBLK*NIDX_BLK//16] i16 and phi mask
    [128, NBLK*NCOL] f16 streams."""
    index_map = np.asarray(index_map).astype(np.int64)
    yp = index_map // WR
    xp = index_map % WR
    idx_full = (1 + yp * 127 + (xp >> 1)).reshape(B, HR, WR)
    phi_full = (xp & 1).reshape(B, HR, WR)

    gidx_cores, mask_cores = [], []
    for k in range(N_CORES):
        b, r0 = k // 4, (k % 4) * SLAB
        idx_blks = np.zeros((NBLK, NG, 256), np.int64)
        phi_blks = np.zeros((NBLK, NG, 256), np.uint8)
        for blk in range(NBLK):
            ys = r0 + blk * NY - 2 + np.arange(NG)
            valid = (ys >= 0) & (ys < HR)
            idx_blks[blk, valid, :WR] = idx_full[b, ys[valid], :]
            phi_blks[blk, valid, :WR] = phi_full[b, ys[valid], :]
        gidx_cores.append(_wrap16(idx_blks.reshape(-1)))
        # M[p, blk*NCOL + g*2 + xc] = phi[blk, g, xc*128+p]
        M = phi_blks.reshape(NBLK, NG, 2, 128).transpose(3, 0, 1, 2)
        mask_cores.append(np.ascontiguousarray(M.reshape(128, NBLK * NCOL)))
    return gidx_cores, mask_cores


def _build_program():
    """Build the SPMD Bacc program (shared by all 8 cores)."""
    import bass_rust
    import concourse.bacc as bacc
    import concourse.tile as tile
    from concourse import mybir
    from concourse.library_config import mlp

    GCOLS = NBLK * NIDX_BLK // 16   # 1152
    MCOLS = NBLK * NCOL             # 144

    nc = bacc.Bacc(
        "TRN2",
        target_bir_lowering=False,
        debug=False,
        enable_asserts=False,
        num_devices=N_CORES,
        num_swdge_queues=NQ,
    )
    tab_t = nc.dram_tensor(
        "tab", [NTAB, ES], mybir.dt.float16, kind="ExternalInput")
    gidx_t = nc.dram_tensor(
        "gidx", [128, GCOLS], mybir.dt.int16, kind="ExternalInput")
    mask_t = nc.dram_tensor(
        "mask", [128, MCOLS], mybir.dt.uint8, kind="ExternalInput")
    wts_t = nc.dram_tensor(
        "wts", [128, 264], mybir.dt.float16, kind="ExternalInput")
    out_t = nc.dram_tensor(
        "out", [2, 128, SLAB, C], mybir.dt.float32, kind="ExternalOutput")

    src_ap = tab_t[:, :].copy()
    src_ap.ap = bass_rust.VecI64Pair([[ES, NTAB], [1, ES]])

    with tile.TileContext(nc) as tc:
        with tc.tile_pool(name="idx", bufs=1) as idxpool, \
             tc.tile_pool(name="g4", bufs=3) as gpool, \
             tc.tile_pool(name="mrg", bufs=2) as mpool, \
             tc.psum_pool(name="ps", bufs=4) as ppool:
            nc.gpsimd.load_library(mlp)

            GI = idxpool.tile([128, GCOLS], mybir.dt.int16)
            nc.sync.dma_start(out=GI[:, 0:GCOLS // NBLK],
                              in_=gidx_t[:, 0:GCOLS // NBLK])
            nc.sync.dma_start(out=GI[:, GCOLS // NBLK:],
                              in_=gidx_t[:, GCOLS // NBLK:])
            M = idxpool.tile([128, MCOLS], mybir.dt.uint8)
            nc.sync.dma_start(out=M[:], in_=mask_t[:, :])
            WT = idxpool.tile([128, 264], mybir.dt.float16)
            nc.sync.dma_start(out=WT[:], in_=wts_t[:, :])

            qn = 0
            for blk in range(NBLK):
                G4 = gpool.tile([128, NCOL, ES], mybir.dt.float16)
                done = 0
                while done < NIDX_BLK:
                    sub = min(SUBI, NIDX_BLK - done)
                    nc.gpsimd.dma_gather(
                        G4[:, done // 128:(done + sub) // 128, :],
                        src_ap,
                        GI[:, (blk * NIDX_BLK + done) // 16:
                           (blk * NIDX_BLK + done + sub) // 16],
                        sub, sub, ES, elem_step=ES,
                        single_packet=False,
                        queue_num=qn % NQ,
                    )
                    qn += 1
                    done += sub

                # per-yh-half phi-compaction: half 0 (g 0..9, cols 0..19)
                # only needs the first 5 sub-gathers, so DVE starts while
                # the rest of the block is still in flight.
                def compact(c0, c1):
                    ncols = c1 - c0
                    mask_ap = M[:, blk * NCOL + c0:blk * NCOL + c1].copy()
                    mask_ap.ap = bass_rust.VecI64Pair(
                        [[MCOLS, 128], [1, ncols], [0, C]])
                    for v in range(3):
                        for i in range(3):
                            out_ap = G4[:, c0:c1, :].copy()
                            out_ap.ap = bass_rust.VecI64Pair(
                                [[NCOL * ES, 128], [ES, ncols], [1, C]])
                            out_ap.offset += i * 256 + v * C
                            dat_ap = G4[:, c0:c1, :].copy()
                            dat_ap.ap = bass_rust.VecI64Pair(
                                [[NCOL * ES, 128], [ES, ncols], [1, C]])
                            dat_ap.offset += i * 256 + (v + 1) * C
                            nc.vector.copy_predicated(out_ap, mask_ap, dat_ap)

                O32 = mpool.tile([128, 2, NY, C], mybir.dt.float32)
                for yh in range(2):
                    compact(0, 20) if yh == 0 else compact(20, NCOL)

                    # fold: out[q=x+v, Yl, c] = sum_iv SH_v @ G4[x, i, v],
                    # PSUM-accumulated per 512-f32 chunk (xc, 8 y-rows).
                    for xc in range(2):
                        P = ppool.tile([128, 8 * C], mybir.dt.float32)
                        ops = []
                        for v in range(3):
                            ops += [(2 - v, 0, v, i) for i in range(3)]
                            if xc == 1 and v > 0:   # 128-boundary wrap
                                ops += [(132 + 4 - v, 1, v, i)
                                        for i in range(3)]
                        for n, (wofs, wrap, v, i) in enumerate(ops):
                            rhs = G4[:, :, :].copy()
                            rhs.ap = bass_rust.VecI64Pair(
                                [[NCOL * ES, 128], [2 * ES, 8], [1, C]])
                            rhs.offset += (
                                (2 - i + yh * 8) * 2 * ES
                                + (xc - wrap) * ES + i * 256 + v * C)
                            nc.tensor.matmul(
                                P[:, :], WT[:, wofs:wofs + 128], rhs,
                                start=(n == 0), stop=(n == len(ops) - 1))
                        nc.scalar.copy(
                            O32[:, xc, yh * 8:(yh + 1) * 8, :], P[:, :])

                # writeout: out_t[xc, p, blk*NY+Yl, c] = O32[p, xc, Yl, c]
                out_ap = out_t[:, :, :, :].copy()
                out_ap.ap = bass_rust.VecI64Pair(
                    [[SLAB * C, 128], [128 * SLAB * C, 2], [C, NY], [1, C]])
                out_ap.offset += blk * NY * C
                nc.sync.dma_start(out=out_ap, in_=O32[:, :, :, :])
    nc.compile()
    return nc


def _build_weights() -> np.ndarray:
    """[128, 264] f16: cols 0..131 banded shift (1 at j=p+2, so
    W_v = wts[:, 2-v:2-v+128] maps partition p -> p+v); cols 132..263
    wrap matrices (1 at j=132+p-124 for p>=124, so Wr_v =
    wts[:, 132+4-v:...] maps p -> p+v-128)."""
    W = np.zeros((128, 264), np.float16)
    p = np.arange(128)
    W[p, p + 2] = 1.0
    pw = np.arange(124, 128)
    W[pw, 132 + pw - 124] = 1.0
    return W


def build(lr, ref, index_map):
    """Host prep: returns (nc, in_maps, assemble) without running."""
    ref = np.ascontiguousarray(np.asarray(ref, dtype=np.float32))
    tabs = _build_tables(ref)
    gidx_cores, mask_cores = _build_streams(index_map)
    wts = _build_weights()
    nc = _build_program()

    in_maps = [
        {
            "tab": tabs[k // 4],
            "gidx": gidx_cores[k],
            "mask": mask_cores[k],
            "wts": wts,
        }
        for k in range(N_CORES)
    ]

    def assemble(results):
        out = np.empty((B, C, HO, WO), np.float32)
        for k in range(N_CORES):
            b, r0 = k // 4, (k % 4) * SLAB
            arr = results[k]["out"]        # [2, 128, SLAB, C]
            # out[b, c, r0+Yl, xc*128+p] = arr[xc, p, Yl, c]
            out[b, :, r0:r0 + SLAB, :] = (
                arr.transpose(3, 2, 0, 1).reshape(C, SLAB, HO))
        return out

    return nc, in_maps, assemble


def kernel(lr, ref, index_map):
    from concourse.bass_utils import run_bass_kernel_spmd

    nc, in_maps, assemble = build(lr, ref, index_map)
    results = []
    for lo in range(0, N_CORES, CORES_PER_LAUNCH):
        ncore = min(CORES_PER_LAUNCH, N_CORES - lo)
        res = run_bass_kernel_spmd(
            nc, in_maps[lo:lo + ncore], list(range(ncore)))
        results.extend(res.results)
    return assemble(results)

